# revision 20
# baseline (speedup 1.0000x reference)
"""ColBERT pairwise + in-batch negative CE loss on 8 Trainium2 NeuronCores.

Problem shapes (hardcoded): B=64, N=32, S=256, D=128, fp32.

reference:
    pos_scores[b]  = sum_n max_s  q[b,n,:] . d[b,s,:]
    neg_scores[b]  = sum_n max_s  q[b,n,:] . neg[b,s,:]
    scores[b,c]    = sum_n max_s  q[b,n,:] . d[c,s,:]
    loss = (mean softplus(neg_scores - pos_scores)
            + mean softplus(max_offdiag_c scores[b,c] - scores[b,b])) / 2

Sharding: the in-batch score matrix is sharded over the doc dim c (8 docs per
core; every core sees all 64*32 query rows).  The pairwise-neg term is
data-parallel over b (8 queries + their neg docs per core).  The host
pre-transposes all operands to d-major layout so the device does zero
transposes; the contraction dim d=128 maps onto the PE partition dim.

Per-core compute (16 chunks of 128 query rows x 2048 local doc cols each):
the only engine that can evacuate PSUM with a max-reduce is the vector
engine at ~1 elem/cycle, which would serialize the whole kernel (~36us).
So the s-max is computed two ways and the work is split across engines:

  DIRECT chunks (6):  DVE segmented reduce_max straight from PSUM.
  LSE chunks (10):    scalar engine evacuates PSUM via exp(2x-70) -> bf16
                      (same cost as a plain copy), the DMA engines' CCE
                      fold the exp'd cols down with accumulate DMAs
                      (SWDGE add), and DVE only runs a short segment-sum
                      tail.  0.5*ln(sum)+35 at the end converts the sums
                      back to max estimates: log-sum-exp with k=2,
                      upper-biased by ln(m_eff)/2 ~ 1e-3 here (gaps
                      between order statistics >> 1/k).

The Act Ln table is only valid for inputs in ~[1e-19, 1e16] but the sums
span up to ~4e32, so Ln runs twice -- once plain, once with scale=1e-16
(= ln(sum) - 36.84) -- and DVE selects per element on sum >= 1e10.

The pairwise term is computed BOTH ways (exact max-reduce + LSE); the host
picks, per 4-query group, whichever matches the treatment of the in-batch
row (LSE biases then cancel to first order in neg - pos).

The in-batch doc columns use a k-blocked doc-minor layout
(col = kblk*256 + c*32 + s_low, s = kblk*32 + s_low) so the CCE folds pair
same-doc columns and every reduce has a contiguous innermost dim.

Emission is software-pipelined: each LSE pair's fold chain (fold1 -> fold2
-> fold3 -> DVE tail) is spread over later chunk steps so no in-order
engine queue ever head-blocks on an unfinished DMA.

Per core the device produces a (4, 132) fp32 tile:
  cols 0..47:    direct chunks (row j, col 8*i + c) for i-th direct chunk
  cols 48..49:   exact pairwise (col 48+g, row j -> local b = 4g+j)
  cols 50..129:  LSE chunks (row j, col 50 + 8*i + c) for i-th LSE chunk
  cols 130..131: LSE pairwise (col 130+g)
The host un-permutes the chunk blocks, assembles the full (64, 64) scores
matrix + the 64 neg pairwise scores and applies the softplus/mean epilogue.
"""

import sys

import numpy as np


def _ensure_path():
    try:
        import concourse  # noqa: F401
    except ImportError:
        sys.path.insert(0, "/opt/trn_rl_repo")


_ensure_path()

import concourse.bacc as bacc  # noqa: E402
import concourse.mybir as mybir  # noqa: E402
from concourse.bass_utils import run_bass_kernel_spmd  # noqa: E402
from concourse.tile import TileContext  # noqa: E402

B, N, S, D = 64, 32, 256, 128
NC = 8
CL = B // NC  # docs / queries per core (8)
BN = B * N  # 2048 query rows
DCOLS = CL * S  # 2048 doc columns per core
NEG_INF_DIAG = 1000000.0

F32 = mybir.dt.float32
F16 = mybir.dt.float16
BF16 = mybir.dt.bfloat16
MMDT = mybir.dt.float16

DIRECT = [2, 5, 8, 11, 14, 15]
LSE = [m for m in range(16) if m not in DIRECT]
PAIRS = [(LSE[2 * p], LSE[2 * p + 1]) for p in range(5)]  # adjacent chunk pairs
# fold depth per pair: late pairs fold less (shorter DMA chains at the tail;
# the vector engine picks up the longer sum-tails instead)
FOLDS = [3, 3, 2, 2, 1]
# V-queue step at which each pair's sum-tail is emitted: late pairs go after
# the m14/m15 direct reduces so the in-order vector queue never head-blocks
# on an unfinished fold DMA.
TAIL_STEP = [8, 11, 14, 16, 17]
K_LSE = 2.0
C_LSE = 70.0
LN_SHIFT = 36.8413614879047  # ln(1e16)

_CACHE = {}


def _install_ntff_shim():
    """Best-effort: register the axon NTFF profile hook so BASS_TRACE=1
    produces hardware profiles.  Safe no-op when unavailable."""
    try:
        import types

        import antenv

        if "antenv.axon_hooks" in sys.modules:
            return
        import trn_agent_boot.trn_boot as tb

        mod = types.ModuleType("antenv.axon_hooks")
        _hook = [None]
        mod.set_axon_ntff_profile_hook = lambda h: _hook.__setitem__(0, h)
        mod.get_axon_ntff_profile_hook = lambda: _hook[0]
        sys.modules["antenv.axon_hooks"] = mod
        antenv.axon_hooks = mod
        mod.set_axon_ntff_profile_hook(
            tb._ntff_profile_via_ctypes("/opt/axon/libaxon_pjrt.so")
        )
    except Exception:
        pass


def _build():
    nc = bacc.Bacc("TRN2", target_bir_lowering=False, debug=False, num_devices=NC)
    qT = nc.dram_tensor("qT", [D, BN], MMDT, kind="ExternalInput")
    dT = nc.dram_tensor("dT", [D, DCOLS], MMDT, kind="ExternalInput")
    nT = nc.dram_tensor("nT", [D, DCOLS], MMDT, kind="ExternalInput")
    qp = nc.dram_tensor("qp", [D, CL * N], MMDT, kind="ExternalInput")
    ones = nc.dram_tensor("ones", [D, 4], F16, kind="ExternalInput")
    out_d = nc.dram_tensor("out", [4, 132], F32, kind="ExternalOutput")

    ADD = mybir.AluOpType.add
    X = mybir.AxisListType.X
    XY = mybir.AxisListType.XY

    with TileContext(nc) as tc:
        with (
            tc.tile_pool(name="sb", bufs=1) as sb,
            tc.tile_pool(name="sc", bufs=4) as sc,
            tc.tile_pool(name="tp", bufs=3) as tp,
            tc.tile_pool(name="ps", bufs=4, space="PSUM") as ps,
        ):
            qs = sb.tile([D, BN], MMDT, tag="qs")
            ds = sb.tile([D, DCOLS], MMDT, tag="ds")
            ns = sb.tile([D, DCOLS], MMDT, tag="ns")
            qps = sb.tile([D, CL * N], MMDT, tag="qps")
            onesb = sb.tile([D, 4], F16, tag="ones")
            maxd = sb.tile([128, 50], F16, tag="maxd")
            maxl = sb.tile([128, 82], F16, tag="maxl")
            stage = sb.tile([128, 82], F32, tag="stage")
            lnA = sb.tile([128, 82], F32, tag="lnA")
            lnB = sb.tile([128, 82], F32, tag="lnB")
            selA = sb.tile([128, 82], F16, tag="selA")
            selB = sb.tile([128, 82], F16, tag="selB")
            mask = sb.tile([128, 82], mybir.dt.uint8, tag="mask")
            biasc = sb.tile([128, 1], F32, tag="biasc")
            pwexp = sb.tile([128, 512], BF16, tag="pwexp")
            outsb = sb.tile([4, 132], F32, tag="outsb")

            nc.gpsimd.memset(biasc[:, :], -C_LSE)

            # Input DMAs in earliest-need order across both HW-DGE queues.
            nc.sync.dma_start(out=ds[:, 0:512], in_=dT[:, 0:512])
            nc.sync.dma_start(out=qs[:, 0:512], in_=qT[:, 0:512])
            nc.sync.dma_start(out=ds[:, 512:1024], in_=dT[:, 512:1024])
            nc.sync.dma_start(out=ds[:, 1024:2048], in_=dT[:, 1024:2048])
            nc.sync.dma_start(out=qs[:, 512:2048], in_=qT[:, 512:2048])
            nc.sync.dma_start(out=qps[:, :], in_=qp[:, :])
            nc.sync.dma_start(out=onesb[:, :], in_=ones[:, :])
            nc.scalar.dma_start(out=ns[:, :], in_=nT[:, :])

            # pair p's scratch: chunk PAIRS[p][0] at cols 0:2048, [1] at
            # 2048:4096 of one [128, 4096] bf16 tile.
            pairbuf = {}

            def fold(p, level):
                # accumulate the upper half of each chunk's live region onto
                # the lower half, for both chunks of the pair at once.  The
                # k-blocked layout (col = kblk*256 + c*32 + s_low) keeps all
                # three halvings doc-aligned.
                pb = pairbuf[p]
                w = 1024 >> level  # 1024, 512, 256
                halves = pb[:, 0:4096].rearrange("p (t x) -> p t x", t=2)
                nc.gpsimd.dma_start(
                    out=halves[:, :, 0:w],
                    in_=halves[:, :, w : 2 * w],
                    accum_op=ADD,
                )

            def tail(p, width):
                # per-doc segment sums of the folded region -> stage cols.
                pb = pairbuf[p]
                live = pb[:, 0:4096].rearrange("p (t x) -> p t x", t=2)[
                    :, :, 0:width
                ]
                nc.vector.reduce_sum(
                    stage[:, 16 * p : 16 * p + 16],
                    live.rearrange("p t (k c s) -> p t c k s", c=8, s=32),
                    axis=XY,
                )

            # deferred-op schedule: step -> list of thunks.  Dependent fold
            # levels are >= 2 chunk-steps apart so no queue head-blocks.
            sched = {}

            def at(step, fn):
                sched.setdefault(step, []).append(fn)

            for p, (c1, c2) in enumerate(PAIRS):
                nf = FOLDS[p]
                for lv in range(nf):
                    at(c2 + 1 + 2 * lv, lambda p=p, lv=lv: fold(p, lv))
                at(TAIL_STEP[p], lambda p=p, nf=nf: tail(p, 2048 >> nf))

            def emit_chunk(m):
                # two [128, 1024] half-tiles (2 PSUM banks each) so 4 chunk
                # halves are in flight: the V/Act PSUM consumers of
                # different chunks overlap instead of gating the PE.
                lse_i = None if m in DIRECT else LSE.index(m)
                if lse_i is not None and lse_i % 2 == 0:
                    pairbuf[lse_i // 2] = sc.tile(
                        [128, 4096], BF16, tag="scratch", name=f"pb{lse_i//2}"
                    )
                tmp = None
                if lse_i is None:
                    tmp = tp.tile([128, 16], F16, tag="tmp", name=f"tmp{m}")
                for h in range(2):
                    t = ps.tile([128, 1024], F32, tag="chunk", name=f"ch{m}_{h}")
                    for u in range(2):
                        c0 = 1024 * h + 512 * u
                        nc.tensor.matmul(
                            t[:, 512 * u : 512 * (u + 1)],
                            qs[:, 128 * m : 128 * (m + 1)],
                            ds[:, c0 : c0 + 512],
                            start=True,
                            stop=True,
                        )
                    if lse_i is None:
                        nc.vector.reduce_max(
                            tmp[:, 8 * h : 8 * h + 8],
                            t[:, :].rearrange("p (k c s) -> p c k s", c=8, s=32),
                            axis=XY,
                        )
                    else:
                        p, slot = divmod(lse_i, 2)
                        nc.scalar.activation(
                            pairbuf[p][
                                :, 2048 * slot + 1024 * h : 2048 * slot + 1024 * h + 1024
                            ],
                            t[:, :],
                            mybir.ActivationFunctionType.Exp,
                            bias=biasc[:, :],
                            scale=K_LSE,
                        )
                if lse_i is None:
                    i = DIRECT.index(m)
                    nc.vector.tensor_max(
                        maxd[:, 8 * i : 8 * i + 8], tmp[:, 0:8], tmp[:, 8:16]
                    )

            def emit_pairwise():
                pt = ps.tile([128, 512], F32, tag="chunk")
                for b in range(CL):
                    g, j = divmod(b, 4)
                    nc.tensor.matmul(
                        pt[32 * j : 32 * (j + 1), 256 * g : 256 * (g + 1)],
                        qps[:, 32 * b : 32 * (b + 1)],
                        ns[:, 256 * b : 256 * (b + 1)],
                        start=True,
                        stop=True,
                        tile_position=(0, 32 * j),
                    )
                nc.vector.reduce_max(
                    maxd[:, 48:50],
                    pt[:, :].rearrange("p (g s) -> p g s", s=S),
                    axis=X,
                )
                nc.scalar.activation(
                    pwexp[:, :],
                    pt[:, :],
                    mybir.ActivationFunctionType.Exp,
                    bias=biasc[:, :],
                    scale=K_LSE,
                )

            def pw_fold():
                nc.gpsimd.dma_start(
                    out=pwexp[:, :].rearrange("p (g s) -> p g s", g=2)[:, :, 0:128],
                    in_=pwexp[:, :].rearrange("p (g s) -> p g s", g=2)[:, :, 128:256],
                    accum_op=ADD,
                )

            def pw_tail():
                nc.vector.reduce_sum(
                    stage[:, 80:82],
                    pwexp[:, :].rearrange("p (g s) -> p g s", g=2)[:, :, 0:128],
                    axis=X,
                )

            at(11, pw_fold)
            at(14, pw_tail)

            for m in range(16):
                emit_chunk(m)
                if m == 8:
                    emit_pairwise()
                for fn in sched.pop(m, []):
                    fn()
            for step in sorted(sched):
                for fn in sched[step]:
                    fn()

            # LSE epilogue: maxl = 0.5*ln(stage) + 35 with a two-range Ln
            # (the Act Ln table is garbage above ~1e16; sums reach ~4e32).
            nc.gpsimd.tensor_scalar(
                mask[:, :], stage[:, :], 1e10, None, op0=mybir.AluOpType.is_ge
            )
            nc.scalar.activation(
                lnA[:, :], stage[:, :], mybir.ActivationFunctionType.Ln
            )
            nc.scalar.activation(
                lnB[:, :],
                stage[:, :],
                mybir.ActivationFunctionType.Ln,
                scale=1e-16,
            )
            nc.gpsimd.tensor_scalar(
                selA[:, :],
                lnA[:, :],
                1.0 / K_LSE,
                C_LSE / K_LSE,
                op0=mybir.AluOpType.mult,
                op1=ADD,
            )
            nc.gpsimd.tensor_scalar(
                selB[:, :],
                lnB[:, :],
                1.0 / K_LSE,
                C_LSE / K_LSE + LN_SHIFT / K_LSE,
                op0=mybir.AluOpType.mult,
                op1=ADD,
            )
            nc.vector.select(maxl[:, :], mask[:, :], selB[:, :], selA[:, :])

            # n-sum via block-ones matmuls: out[j, col] = sum_n max[32j+n, col]
            ot = ps.tile([4, 132], F32, tag="chunk")
            nc.tensor.matmul(
                ot[:, 50:132], onesb[:, :], maxl[:, :], start=True, stop=True
            )
            nc.tensor.matmul(
                ot[:, 0:50], onesb[:, :], maxd[:, :], start=True, stop=True
            )
            nc.scalar.activation(
                outsb[:, :], ot[:, :], mybir.ActivationFunctionType.Copy
            )
            nc.sync.dma_start(out=out_d[:, :], in_=outsb[:, :])

    nc.finalize()
    return nc


LAST_RESULT = None


def kernel(query_embeddings, doc_embeddings, neg_doc_embeddings):
    global LAST_RESULT
    _install_ntff_shim()

    q = np.asarray(query_embeddings, dtype=np.float32)
    d = np.asarray(doc_embeddings, dtype=np.float32)
    g = np.asarray(neg_doc_embeddings, dtype=np.float32)
    assert q.shape == (B, N, D) and d.shape == (B, S, D) and g.shape == (B, S, D)

    qT_all = np.ascontiguousarray(q.transpose(2, 0, 1).reshape(D, BN).astype(np.float16))
    ones_blk = np.zeros((D, 4), dtype=np.float16)
    ones_blk[np.arange(D), np.arange(D) // 32] = 1.0

    in_maps = []
    for k in range(NC):
        # in-batch docs: k-blocked doc-minor (col = kblk*256 + c*32 + s_low)
        dk = d[CL * k : CL * (k + 1)]  # [8, 256, 128]
        dT_k = np.ascontiguousarray(
            dk.transpose(2, 1, 0)  # [D, S, C]
            .reshape(D, 8, 32, CL)  # [D, kblk, s_low, c]
            .transpose(0, 1, 3, 2)  # [D, kblk, c, s_low]
            .reshape(D, DCOLS)
            .astype(np.float16)
        )
        nT_k = np.ascontiguousarray(
            g[CL * k : CL * (k + 1)].transpose(2, 0, 1).reshape(D, DCOLS).astype(np.float16)
        )
        qp_k = np.ascontiguousarray(qT_all[:, CL * N * k : CL * N * (k + 1)])
        in_maps.append(
            {"qT": qT_all, "dT": dT_k, "nT": nT_k, "qp": qp_k, "ones": ones_blk}
        )

    if "nc" not in _CACHE:
        _CACHE["nc"] = _build()
    res = run_bass_kernel_spmd(_CACHE["nc"], in_maps, core_ids=list(range(NC)))
    LAST_RESULT = res

    scores = np.empty((B, B), dtype=np.float32)
    negpair = np.empty((B,), dtype=np.float32)
    for k in range(NC):
        o = res.results[k]["out"]  # (4, 132)
        o_full = np.empty((4, 128), dtype=np.float32)
        for i, m in enumerate(DIRECT):
            o_full[:, 8 * m : 8 * m + 8] = o[:, 8 * i : 8 * i + 8]
        for i, m in enumerate(LSE):
            o_full[:, 8 * m : 8 * m + 8] = o[:, 50 + 8 * i : 58 + 8 * i]
        scores[:, CL * k : CL * (k + 1)] = (
            o_full.reshape(4, 16, CL).transpose(1, 0, 2).reshape(B, CL)
        )
        # negpair group g covers local b = 4g+j <-> global chunk 2k+g; pick
        # the flavor matching that chunk's in-batch treatment so LSE biases
        # cancel in neg - pos.
        for gcol in range(2):
            col = 48 + gcol if (2 * k + gcol) in DIRECT else 130 + gcol
            for j in range(4):
                negpair[CL * k + 4 * gcol + j] = o[j, col]

    pos = np.diagonal(scores).astype(np.float64)
    l1 = np.logaddexp(0.0, negpair.astype(np.float64) - pos).mean()
    neg_ib = (
        scores.astype(np.float64) - np.eye(B, dtype=np.float64) * NEG_INF_DIAG
    ).max(axis=1)
    l2 = np.logaddexp(0.0, neg_ib - pos).mean()
    return np.asarray((l1 + l2) / 2.0, dtype=np.float32)


# revision 21
# speedup vs baseline: 1.0431x; 1.0431x over previous
"""ColBERT pairwise + in-batch negative CE loss on 8 Trainium2 NeuronCores.

Problem shapes (hardcoded): B=64, N=32, S=256, D=128, fp32.

reference:
    pos_scores[b]  = sum_n max_s  q[b,n,:] . d[b,s,:]
    neg_scores[b]  = sum_n max_s  q[b,n,:] . neg[b,s,:]
    scores[b,c]    = sum_n max_s  q[b,n,:] . d[c,s,:]
    loss = (mean softplus(neg_scores - pos_scores)
            + mean softplus(max_offdiag_c scores[b,c] - scores[b,b])) / 2

Sharding: the in-batch score matrix is sharded over the doc dim c (8 docs per
core; every core sees all 64*32 query rows).  The pairwise-neg term is
data-parallel over b (8 queries + their neg docs per core).  The host
pre-transposes all operands to d-major layout so the device does zero
transposes; the contraction dim d=128 maps onto the PE partition dim.

Per-core compute (16 chunks of 128 query rows x 2048 local doc cols each):
the only engine that can evacuate PSUM with a max-reduce is the vector
engine at ~1 elem/cycle, which would serialize the whole kernel (~36us).
So the s-max is computed two ways and the work is split across engines:

  DIRECT chunks (6):  DVE segmented reduce_max straight from PSUM.
  LSE chunks (10):    scalar engine evacuates PSUM via exp(2x-70) -> bf16
                      (same cost as a plain copy), the DMA engines' CCE
                      fold the exp'd cols down with accumulate DMAs
                      (SWDGE add), and DVE only runs a short segment-sum
                      tail.  0.5*ln(sum)+35 at the end converts the sums
                      back to max estimates: log-sum-exp with k=2,
                      upper-biased by ln(m_eff)/2 ~ 1e-3 here (gaps
                      between order statistics >> 1/k).

The Act Ln table is only valid for inputs in ~[1e-19, 1e16] but the sums
span up to ~4e32, so Ln runs twice -- once plain, once with scale=1e-16
(= ln(sum) - 36.84) -- and DVE selects per element on sum >= 1e10.

The pairwise term is computed BOTH ways (exact max-reduce + LSE); the host
picks, per 4-query group, whichever matches the treatment of the in-batch
row (LSE biases then cancel to first order in neg - pos).

The in-batch doc columns use a k-blocked doc-minor layout
(col = kblk*256 + c*32 + s_low, s = kblk*32 + s_low) so the CCE folds pair
same-doc columns and every reduce has a contiguous innermost dim.

Emission is software-pipelined: each LSE pair's fold chain (fold1 -> fold2
-> fold3 -> DVE tail) is spread over later chunk steps so no in-order
engine queue ever head-blocks on an unfinished DMA.

Per core the device produces a (4, 132) fp32 tile:
  cols 0..47:    direct chunks (row j, col 8*i + c) for i-th direct chunk
  cols 48..49:   exact pairwise (col 48+g, row j -> local b = 4g+j)
  cols 50..129:  LSE chunks (row j, col 50 + 8*i + c) for i-th LSE chunk
  cols 130..131: LSE pairwise (col 130+g)
The host un-permutes the chunk blocks, assembles the full (64, 64) scores
matrix + the 64 neg pairwise scores and applies the softplus/mean epilogue.
"""

import sys

import numpy as np


def _ensure_path():
    try:
        import concourse  # noqa: F401
    except ImportError:
        sys.path.insert(0, "/opt/trn_rl_repo")


_ensure_path()

import concourse.bacc as bacc  # noqa: E402
import concourse.mybir as mybir  # noqa: E402
from concourse.bass_utils import run_bass_kernel_spmd  # noqa: E402
from concourse.tile import TileContext  # noqa: E402

B, N, S, D = 64, 32, 256, 128
NC = 8
CL = B // NC  # docs / queries per core (8)
BN = B * N  # 2048 query rows
DCOLS = CL * S  # 2048 doc columns per core
NEG_INF_DIAG = 1000000.0

F32 = mybir.dt.float32
F16 = mybir.dt.float16
BF16 = mybir.dt.bfloat16
MMDT = mybir.dt.float16

DIRECT = [2, 5, 8, 11, 14, 15]
LSE = [m for m in range(16) if m not in DIRECT]
PAIRS = [(LSE[2 * p], LSE[2 * p + 1]) for p in range(5)]  # adjacent chunk pairs
# fold depth per pair: late pairs fold less (shorter DMA chains at the tail;
# the vector engine picks up the longer sum-tails instead)
FOLDS = [3, 3, 2, 2, 1]
# V-queue step at which each pair's sum-tail is emitted: late pairs go after
# the m14/m15 direct reduces so the in-order vector queue never head-blocks
# on an unfinished fold DMA.
TAIL_STEP = [8, 11, 14, 16, 17]
K_LSE = 2.0
C_LSE = 70.0
LN_SHIFT = 36.8413614879047  # ln(1e16)

_CACHE = {}


def _install_ntff_shim():
    """Best-effort: register the axon NTFF profile hook so BASS_TRACE=1
    produces hardware profiles.  Safe no-op when unavailable."""
    try:
        import types

        import antenv

        if "antenv.axon_hooks" in sys.modules:
            return
        import trn_agent_boot.trn_boot as tb

        mod = types.ModuleType("antenv.axon_hooks")
        _hook = [None]
        mod.set_axon_ntff_profile_hook = lambda h: _hook.__setitem__(0, h)
        mod.get_axon_ntff_profile_hook = lambda: _hook[0]
        sys.modules["antenv.axon_hooks"] = mod
        antenv.axon_hooks = mod
        mod.set_axon_ntff_profile_hook(
            tb._ntff_profile_via_ctypes("/opt/axon/libaxon_pjrt.so")
        )
    except Exception:
        pass


def _build():
    nc = bacc.Bacc("TRN2", target_bir_lowering=False, debug=False, num_devices=NC)
    qT = nc.dram_tensor("qT", [D, BN], MMDT, kind="ExternalInput")
    dT = nc.dram_tensor("dT", [D, DCOLS], MMDT, kind="ExternalInput")
    nT = nc.dram_tensor("nT", [D, DCOLS], MMDT, kind="ExternalInput")
    qp = nc.dram_tensor("qp", [D, CL * N], MMDT, kind="ExternalInput")
    ones = nc.dram_tensor("ones", [D, 4], F16, kind="ExternalInput")
    out_d = nc.dram_tensor("out", [4, 132], F32, kind="ExternalOutput")

    ADD = mybir.AluOpType.add
    X = mybir.AxisListType.X
    XY = mybir.AxisListType.XY

    with TileContext(nc) as tc:
        with (
            tc.tile_pool(name="sb", bufs=1) as sb,
            tc.tile_pool(name="sc", bufs=4) as sc,
            tc.tile_pool(name="tp", bufs=3) as tp,
            tc.tile_pool(name="ps", bufs=4, space="PSUM") as ps,
        ):
            qs = sb.tile([D, BN], MMDT, tag="qs")
            ds = sb.tile([D, DCOLS], MMDT, tag="ds")
            ns = sb.tile([D, DCOLS], MMDT, tag="ns")
            qps = sb.tile([D, CL * N], MMDT, tag="qps")
            onesb = sb.tile([D, 4], F16, tag="ones")
            maxd = sb.tile([128, 50], F16, tag="maxd")
            maxl = sb.tile([128, 82], F16, tag="maxl")
            stage = sb.tile([128, 82], F32, tag="stage")
            lnA = sb.tile([128, 82], F32, tag="lnA")
            lnB = sb.tile([128, 82], F32, tag="lnB")
            selA = sb.tile([128, 82], F16, tag="selA")
            selB = sb.tile([128, 82], F16, tag="selB")
            mask = sb.tile([128, 82], mybir.dt.uint8, tag="mask")
            biasc = sb.tile([128, 1], F32, tag="biasc")
            pwexp = sb.tile([128, 512], BF16, tag="pwexp")
            outsb = sb.tile([4, 132], F32, tag="outsb")

            nc.gpsimd.memset(biasc[:, :], -C_LSE)

            # Input DMAs in earliest-need order across both HW-DGE queues.
            nc.sync.dma_start(out=ds[:, 0:512], in_=dT[:, 0:512])
            nc.sync.dma_start(out=qs[:, 0:512], in_=qT[:, 0:512])
            nc.sync.dma_start(out=ds[:, 512:1024], in_=dT[:, 512:1024])
            nc.sync.dma_start(out=ds[:, 1024:2048], in_=dT[:, 1024:2048])
            nc.sync.dma_start(out=qs[:, 512:2048], in_=qT[:, 512:2048])
            nc.sync.dma_start(out=qps[:, :], in_=qp[:, :])
            nc.sync.dma_start(out=onesb[:, :], in_=ones[:, :])
            nc.scalar.dma_start(out=ns[:, :], in_=nT[:, :])

            # pair p's scratch: chunk PAIRS[p][0] at cols 0:2048, [1] at
            # 2048:4096 of one [128, 4096] bf16 tile.
            pairbuf = {}

            def fold(p, level):
                # accumulate the upper half of each chunk's live region onto
                # the lower half, for both chunks of the pair at once.  The
                # k-blocked layout (col = kblk*256 + c*32 + s_low) keeps all
                # three halvings doc-aligned.
                pb = pairbuf[p]
                w = 1024 >> level  # 1024, 512, 256
                halves = pb[:, 0:4096].rearrange("p (t x) -> p t x", t=2)
                nc.gpsimd.dma_start(
                    out=halves[:, :, 0:w],
                    in_=halves[:, :, w : 2 * w],
                    accum_op=ADD,
                )

            def tail(p, width):
                # per-doc segment sums of the folded region -> stage cols.
                pb = pairbuf[p]
                live = pb[:, 0:4096].rearrange("p (t x) -> p t x", t=2)[
                    :, :, 0:width
                ]
                nc.vector.reduce_sum(
                    stage[:, 16 * p : 16 * p + 16],
                    live.rearrange("p t (k c s) -> p t c k s", c=8, s=32),
                    axis=XY,
                )

            # deferred-op schedule: step -> list of thunks.  Dependent fold
            # levels are >= 2 chunk-steps apart so no queue head-blocks.
            sched = {}

            def at(step, fn):
                sched.setdefault(step, []).append(fn)

            for p, (c1, c2) in enumerate(PAIRS):
                nf = FOLDS[p]
                for lv in range(nf):
                    at(c2 + 1 + 2 * lv, lambda p=p, lv=lv: fold(p, lv))
                at(TAIL_STEP[p], lambda p=p, nf=nf: tail(p, 2048 >> nf))

            def emit_chunk(m):
                # two [128, 1024] half-tiles (2 PSUM banks each) so 4 chunk
                # halves are in flight: the V/Act PSUM consumers of
                # different chunks overlap instead of gating the PE.
                lse_i = None if m in DIRECT else LSE.index(m)
                if lse_i is not None and lse_i % 2 == 0:
                    pairbuf[lse_i // 2] = sc.tile(
                        [128, 4096], BF16, tag="scratch", name=f"pb{lse_i//2}"
                    )
                tmp = None
                if lse_i is None:
                    tmp = tp.tile([128, 16], F16, tag="tmp", name=f"tmp{m}")
                for h in range(2):
                    t = ps.tile([128, 1024], F32, tag="chunk", name=f"ch{m}_{h}")
                    for u in range(2):
                        c0 = 1024 * h + 512 * u
                        nc.tensor.matmul(
                            t[:, 512 * u : 512 * (u + 1)],
                            qs[:, 128 * m : 128 * (m + 1)],
                            ds[:, c0 : c0 + 512],
                            start=True,
                            stop=True,
                        )
                    if lse_i is None:
                        nc.vector.reduce_max(
                            tmp[:, 8 * h : 8 * h + 8],
                            t[:, :].rearrange("p (k c s) -> p c k s", c=8, s=32),
                            axis=XY,
                        )
                    else:
                        p, slot = divmod(lse_i, 2)
                        nc.scalar.activation(
                            pairbuf[p][
                                :, 2048 * slot + 1024 * h : 2048 * slot + 1024 * h + 1024
                            ],
                            t[:, :],
                            mybir.ActivationFunctionType.Exp,
                            bias=biasc[:, :],
                            scale=K_LSE,
                        )
                if lse_i is None:
                    i = DIRECT.index(m)
                    nc.vector.tensor_max(
                        maxd[:, 8 * i : 8 * i + 8], tmp[:, 0:8], tmp[:, 8:16]
                    )

            def emit_pairwise():
                pt = ps.tile([128, 512], F32, tag="chunk")
                for b in range(CL):
                    g, j = divmod(b, 4)
                    nc.tensor.matmul(
                        pt[32 * j : 32 * (j + 1), 256 * g : 256 * (g + 1)],
                        qps[:, 32 * b : 32 * (b + 1)],
                        ns[:, 256 * b : 256 * (b + 1)],
                        start=True,
                        stop=True,
                        tile_position=(0, 32 * j),
                    )
                nc.vector.reduce_max(
                    maxd[:, 48:50],
                    pt[:, :].rearrange("p (g s) -> p g s", s=S),
                    axis=X,
                )
                nc.scalar.activation(
                    pwexp[:, :],
                    pt[:, :],
                    mybir.ActivationFunctionType.Exp,
                    bias=biasc[:, :],
                    scale=K_LSE,
                )

            def pw_fold():
                nc.gpsimd.dma_start(
                    out=pwexp[:, :].rearrange("p (g s) -> p g s", g=2)[:, :, 0:128],
                    in_=pwexp[:, :].rearrange("p (g s) -> p g s", g=2)[:, :, 128:256],
                    accum_op=ADD,
                )

            def pw_tail():
                nc.vector.reduce_sum(
                    stage[:, 80:82],
                    pwexp[:, :].rearrange("p (g s) -> p g s", g=2)[:, :, 0:128],
                    axis=X,
                )

            at(11, pw_fold)
            at(14, pw_tail)

            for m in range(16):
                emit_chunk(m)
                if m == 8:
                    emit_pairwise()
                for fn in sched.pop(m, []):
                    fn()
            for step in sorted(sched):
                for fn in sched[step]:
                    fn()

            # LSE epilogue: maxl = 0.5*ln(stage) + 35 with a two-range Ln
            # (the Act Ln table is garbage above ~1e16; sums reach ~4e32).
            nc.vector.tensor_scalar(
                mask[:, :], stage[:, :], 1e10, None, op0=mybir.AluOpType.is_ge
            )
            nc.scalar.activation(
                lnA[:, :], stage[:, :], mybir.ActivationFunctionType.Ln
            )
            nc.scalar.activation(
                lnB[:, :],
                stage[:, :],
                mybir.ActivationFunctionType.Ln,
                scale=1e-16,
            )
            nc.vector.tensor_scalar(
                selA[:, :],
                lnA[:, :],
                1.0 / K_LSE,
                C_LSE / K_LSE,
                op0=mybir.AluOpType.mult,
                op1=ADD,
            )
            nc.vector.tensor_scalar(
                selB[:, :],
                lnB[:, :],
                1.0 / K_LSE,
                C_LSE / K_LSE + LN_SHIFT / K_LSE,
                op0=mybir.AluOpType.mult,
                op1=ADD,
            )
            nc.vector.select(maxl[:, :], mask[:, :], selB[:, :], selA[:, :])

            # n-sum via block-ones matmuls: out[j, col] = sum_n max[32j+n, col]
            ot = ps.tile([4, 132], F32, tag="chunk")
            nc.tensor.matmul(
                ot[:, 50:132], onesb[:, :], maxl[:, :], start=True, stop=True
            )
            nc.tensor.matmul(
                ot[:, 0:50], onesb[:, :], maxd[:, :], start=True, stop=True
            )
            nc.vector.tensor_copy(outsb[:, :], ot[:, :])
            nc.sync.dma_start(out=out_d[:, :], in_=outsb[:, :])

    nc.finalize()
    return nc


LAST_RESULT = None


def kernel(query_embeddings, doc_embeddings, neg_doc_embeddings):
    global LAST_RESULT
    _install_ntff_shim()

    q = np.asarray(query_embeddings, dtype=np.float32)
    d = np.asarray(doc_embeddings, dtype=np.float32)
    g = np.asarray(neg_doc_embeddings, dtype=np.float32)
    assert q.shape == (B, N, D) and d.shape == (B, S, D) and g.shape == (B, S, D)

    qT_all = np.ascontiguousarray(q.transpose(2, 0, 1).reshape(D, BN).astype(np.float16))
    ones_blk = np.zeros((D, 4), dtype=np.float16)
    ones_blk[np.arange(D), np.arange(D) // 32] = 1.0

    in_maps = []
    for k in range(NC):
        # in-batch docs: k-blocked doc-minor (col = kblk*256 + c*32 + s_low)
        dk = d[CL * k : CL * (k + 1)]  # [8, 256, 128]
        dT_k = np.ascontiguousarray(
            dk.transpose(2, 1, 0)  # [D, S, C]
            .reshape(D, 8, 32, CL)  # [D, kblk, s_low, c]
            .transpose(0, 1, 3, 2)  # [D, kblk, c, s_low]
            .reshape(D, DCOLS)
            .astype(np.float16)
        )
        nT_k = np.ascontiguousarray(
            g[CL * k : CL * (k + 1)].transpose(2, 0, 1).reshape(D, DCOLS).astype(np.float16)
        )
        qp_k = np.ascontiguousarray(qT_all[:, CL * N * k : CL * N * (k + 1)])
        in_maps.append(
            {"qT": qT_all, "dT": dT_k, "nT": nT_k, "qp": qp_k, "ones": ones_blk}
        )

    if "nc" not in _CACHE:
        _CACHE["nc"] = _build()
    res = run_bass_kernel_spmd(_CACHE["nc"], in_maps, core_ids=list(range(NC)))
    LAST_RESULT = res

    scores = np.empty((B, B), dtype=np.float32)
    negpair = np.empty((B,), dtype=np.float32)
    for k in range(NC):
        o = res.results[k]["out"]  # (4, 132)
        o_full = np.empty((4, 128), dtype=np.float32)
        for i, m in enumerate(DIRECT):
            o_full[:, 8 * m : 8 * m + 8] = o[:, 8 * i : 8 * i + 8]
        for i, m in enumerate(LSE):
            o_full[:, 8 * m : 8 * m + 8] = o[:, 50 + 8 * i : 58 + 8 * i]
        scores[:, CL * k : CL * (k + 1)] = (
            o_full.reshape(4, 16, CL).transpose(1, 0, 2).reshape(B, CL)
        )
        # negpair group g covers local b = 4g+j <-> global chunk 2k+g; pick
        # the flavor matching that chunk's in-batch treatment so LSE biases
        # cancel in neg - pos.
        for gcol in range(2):
            col = 48 + gcol if (2 * k + gcol) in DIRECT else 130 + gcol
            for j in range(4):
                negpair[CL * k + 4 * gcol + j] = o[j, col]

    pos = np.diagonal(scores).astype(np.float64)
    l1 = np.logaddexp(0.0, negpair.astype(np.float64) - pos).mean()
    neg_ib = (
        scores.astype(np.float64) - np.eye(B, dtype=np.float64) * NEG_INF_DIAG
    ).max(axis=1)
    l2 = np.logaddexp(0.0, neg_ib - pos).mean()
    return np.asarray((l1 + l2) / 2.0, dtype=np.float32)


# revision 22
# speedup vs baseline: 1.1659x; 1.1178x over previous
"""ColBERT pairwise + in-batch negative CE loss on 8 Trainium2 NeuronCores.

Problem shapes (hardcoded): B=64, N=32, S=256, D=128, fp32.

reference:
    pos_scores[b]  = sum_n max_s  q[b,n,:] . d[b,s,:]
    neg_scores[b]  = sum_n max_s  q[b,n,:] . neg[b,s,:]
    scores[b,c]    = sum_n max_s  q[b,n,:] . d[c,s,:]
    loss = (mean softplus(neg_scores - pos_scores)
            + mean softplus(max_offdiag_c scores[b,c] - scores[b,b])) / 2

Sharding: the in-batch score matrix is sharded over the doc dim c (8 docs per
core; every core sees all 64*32 query rows).  The pairwise-neg term is
data-parallel over b (8 queries + their neg docs per core).  The host
pre-transposes all operands to d-major layout so the device does zero
transposes; the contraction dim d=128 maps exactly onto the PE partition dim.

Per core the device produces a (4, 130) fp32 tile:
  cols 0..127:  col 8*m+c, row j  ->  sum_n max_s (q[4m+j] . d_local[c])
  cols 128/129: col 128+g, row j  ->  neg_scores for local b = 4g+j
The host assembles the full (64, 64) scores matrix + the 64 neg pairwise
scores and applies the trivial softplus/mean epilogue (128 scalars).
"""

import sys

import numpy as np


def _ensure_path():
    try:
        import concourse  # noqa: F401
    except ImportError:
        sys.path.insert(0, "/opt/trn_rl_repo")


_ensure_path()

import concourse.bacc as bacc  # noqa: E402
import concourse.mybir as mybir  # noqa: E402
from concourse.bass_utils import run_bass_kernel_spmd  # noqa: E402
from concourse.tile import TileContext  # noqa: E402

B, N, S, D = 64, 32, 256, 128
NC = 8
CL = B // NC  # docs / queries per core (8)
BN = B * N  # 2048 query rows
DCOLS = CL * S  # 2048 doc columns per core
NEG_INF_DIAG = 1000000.0

F32 = mybir.dt.float32
F16 = mybir.dt.float16
MMDT = mybir.dt.float16  # dtype used by the matmul operands

_CACHE = {}


def _install_ntff_shim():
    """Best-effort: register the axon NTFF profile hook so BASS_TRACE=1
    produces hardware profiles.  Safe no-op when unavailable."""
    try:
        import types

        import antenv

        if "antenv.axon_hooks" in sys.modules:
            return
        import trn_agent_boot.trn_boot as tb

        mod = types.ModuleType("antenv.axon_hooks")
        _hook = [None]
        mod.set_axon_ntff_profile_hook = lambda h: _hook.__setitem__(0, h)
        mod.get_axon_ntff_profile_hook = lambda: _hook[0]
        sys.modules["antenv.axon_hooks"] = mod
        antenv.axon_hooks = mod
        mod.set_axon_ntff_profile_hook(
            tb._ntff_profile_via_ctypes("/opt/axon/libaxon_pjrt.so")
        )
    except Exception:
        pass


def _build():
    nc = bacc.Bacc("TRN2", target_bir_lowering=False, debug=False, num_devices=NC)
    qT = nc.dram_tensor("qT", [D, BN], MMDT, kind="ExternalInput")
    dT = nc.dram_tensor("dT", [D, DCOLS], MMDT, kind="ExternalInput")
    nT = nc.dram_tensor("nT", [D, DCOLS], MMDT, kind="ExternalInput")
    qp = nc.dram_tensor("qp", [D, CL * N], MMDT, kind="ExternalInput")
    ones = nc.dram_tensor("ones", [D, 4], F16, kind="ExternalInput")
    out_d = nc.dram_tensor("out", [4, 130], F32, kind="ExternalOutput")

    with TileContext(nc) as tc:
        with (
            tc.tile_pool(name="sb", bufs=1) as sb,
            tc.tile_pool(name="ps", bufs=2, space="PSUM") as ps,
        ):
            qs = sb.tile([D, BN], MMDT, tag="qs")
            ds = sb.tile([D, DCOLS], MMDT, tag="ds")
            ns = sb.tile([D, DCOLS], MMDT, tag="ns")
            qps = sb.tile([D, CL * N], MMDT, tag="qps")
            onesb = sb.tile([D, 4], F16, tag="ones")
            maxall = sb.tile([128, 130], F16, tag="maxall")
            outsb = sb.tile([4, 130], F32, tag="outsb")

            # DMA order: graduated piece sizes so chunk m=0 (qs cols 0:128,
            # ds cols 0:1024) can start after ~300KB instead of the full load.
            nc.sync.dma_start(out=qs[:, 0:128], in_=qT[:, 0:128])
            nc.sync.dma_start(out=ds[:, 0:512], in_=dT[:, 0:512])
            nc.sync.dma_start(out=ds[:, 512:1024], in_=dT[:, 512:1024])
            nc.sync.dma_start(out=qs[:, 128:1024], in_=qT[:, 128:1024])
            nc.sync.dma_start(out=ds[:, 1024:2048], in_=dT[:, 1024:2048])
            nc.sync.dma_start(out=qs[:, 1024:2048], in_=qT[:, 1024:2048])
            for p in range(4):
                sl = slice(512 * p, 512 * (p + 1))
                nc.sync.dma_start(out=ns[:, sl], in_=nT[:, sl])
            nc.sync.dma_start(out=qps[:, :], in_=qp[:, :])
            nc.sync.dma_start(out=onesb[:, :], in_=ones[:, :])

            # In-batch term: query chunk m (128 rows) x all 2048 local doc
            # cols.  m=0 is split in two half-width tiles so the first
            # reduce only gates on ds[0:1024]; the rest use full tiles.
            for m in range(16):
                if m == 0:
                    for h in range(2):
                        t = ps.tile([128, 1024], F32, tag="chunk")
                        for u in range(2):
                            c0 = 1024 * h + 512 * u
                            nc.tensor.matmul(
                                t[:, 512 * u : 512 * (u + 1)],
                                qs[:, 0:128],
                                ds[:, c0 : c0 + 512],
                                start=True,
                                stop=True,
                            )
                        nc.vector.reduce_max(
                            maxall[:, 4 * h : 4 * h + 4],
                            t[:, :].rearrange("p (g s) -> p g s", s=S),
                            axis=mybir.AxisListType.X,
                        )
                    continue
                t = ps.tile([128, 2048], F32, tag="chunk")
                for u in range(4):
                    nc.tensor.matmul(
                        t[:, 512 * u : 512 * (u + 1)],
                        qs[:, 128 * m : 128 * (m + 1)],
                        ds[:, 512 * u : 512 * (u + 1)],
                        start=True,
                        stop=True,
                    )
                nc.vector.reduce_max(
                    maxall[:, 8 * m : 8 * m + 8],
                    t[:, :].rearrange("p (g s) -> p g s", s=S),
                    axis=mybir.AxisListType.X,
                )

            # Pairwise neg term: 8 small matmuls (M=32) col-packed 4-way via
            # tile_position into ONE (128, 512) tile; a single segmented
            # reduce writes maxall[:, 128:130] (local b at partitions
            # 32*(b%4) + n, column 128 + b//4).
            pt = ps.tile([128, 512], F32, tag="chunk")
            for b in range(CL):
                g, j = divmod(b, 4)
                nc.tensor.matmul(
                    pt[32 * j : 32 * (j + 1), 256 * g : 256 * (g + 1)],
                    qps[:, 32 * b : 32 * (b + 1)],
                    ns[:, 256 * b : 256 * (b + 1)],
                    start=True,
                    stop=True,
                    tile_position=(0, 32 * j),
                )
            nc.vector.reduce_max(
                maxall[:, 128:130],
                pt[:, :].rearrange("p (g s) -> p g s", s=S),
                axis=mybir.AxisListType.X,
            )

            # n-sum via block-ones matmul: out[j, col] = sum_{n} maxall[32j+n, col]
            # Split at col 64 (chunks m<8 finish first) so the first half of
            # the epilogue overlaps the remaining reduces.
            for c0, c1 in ((0, 64), (64, 130)):
                ot = ps.tile([4, 130], F32, tag="chunk")
                nc.tensor.matmul(
                    ot[:, 0 : c1 - c0],
                    onesb[:, :],
                    maxall[:, c0:c1],
                    start=True,
                    stop=True,
                )
                nc.vector.tensor_copy(outsb[:, c0:c1], ot[:, 0 : c1 - c0])
                nc.sync.dma_start(out=out_d[:, c0:c1], in_=outsb[:, c0:c1])

    nc.finalize()
    return nc


LAST_RESULT = None


def kernel(query_embeddings, doc_embeddings, neg_doc_embeddings):
    global LAST_RESULT
    _install_ntff_shim()

    q = np.asarray(query_embeddings, dtype=np.float32)
    d = np.asarray(doc_embeddings, dtype=np.float32)
    g = np.asarray(neg_doc_embeddings, dtype=np.float32)
    assert q.shape == (B, N, D) and d.shape == (B, S, D) and g.shape == (B, S, D)

    # d-major layouts
    qT_all = np.ascontiguousarray(q.transpose(2, 0, 1).reshape(D, BN).astype(np.float16))
    ones_blk = np.zeros((D, 4), dtype=np.float16)
    ones_blk[np.arange(D), np.arange(D) // 32] = 1.0

    in_maps = []
    for k in range(NC):
        dT_k = np.ascontiguousarray(
            d[CL * k : CL * (k + 1)].transpose(2, 0, 1).reshape(D, DCOLS).astype(np.float16)
        )
        nT_k = np.ascontiguousarray(
            g[CL * k : CL * (k + 1)].transpose(2, 0, 1).reshape(D, DCOLS).astype(np.float16)
        )
        qp_k = np.ascontiguousarray(qT_all[:, CL * N * k : CL * N * (k + 1)])
        in_maps.append(
            {"qT": qT_all, "dT": dT_k, "nT": nT_k, "qp": qp_k, "ones": ones_blk}
        )

    if "nc" not in _CACHE:
        _CACHE["nc"] = _build()
    res = run_bass_kernel_spmd(_CACHE["nc"], in_maps, core_ids=list(range(NC)))
    LAST_RESULT = res

    # Assemble: scores (64, 64) and pairwise neg scores (64,)
    scores = np.empty((B, B), dtype=np.float32)
    negpair = np.empty((B,), dtype=np.float32)
    for k in range(NC):
        o = res.results[k]["out"]  # (4, 130)
        scores[:, CL * k : CL * (k + 1)] = (
            o[:, :128].reshape(4, 16, CL).transpose(1, 0, 2).reshape(B, CL)
        )
        for gcol in range(2):
            for j in range(4):
                negpair[CL * k + 4 * gcol + j] = o[j, 128 + gcol]

    pos = np.diagonal(scores).astype(np.float64)
    l1 = np.logaddexp(0.0, negpair.astype(np.float64) - pos).mean()
    neg_ib = (
        scores.astype(np.float64) - np.eye(B, dtype=np.float64) * NEG_INF_DIAG
    ).max(axis=1)
    l2 = np.logaddexp(0.0, neg_ib - pos).mean()
    return np.asarray((l1 + l2) / 2.0, dtype=np.float32)
